# revision 12
# baseline (speedup 1.0000x reference)
"""Bidirectional LSTM (NLReader) Trainium2 Bass kernel.

Strategy (8 NeuronCores, SPMD — one program, per-core inputs differ):
  cores 0-3: forward direction, batch quarters 0..3 (B_local=16)
  cores 4-7: backward direction (inputs time-reversed on host), quarters 0..3

Per core:
  1. Precompute xw[t,b,:] = emb[t,b,:] @ Wih^T + bias  (fp16 matmul, fp32 psum),
     with the padding mask folded in as -60 added to the i,f gate pre-activations
     at padded positions (sigmoid(-60)=0 reproduces h,c zeroing exactly).
     Stored to a DRAM scratch [L*BL, 2048] fp32.
  2. 128 sequential LSTM steps: gates = xw_t (identity-inject matmul, float32r)
     + h_{t-1} @ Whh^T (fp16 stationary h^T, streamed fp16 Whh^T);
     tanh/sigmoid on ScalarE, products on VectorE, h transposed back via PE.
  Gate order in all weight/psum layouts: [g | i | f | o].

kernel(**inputs) takes the FULL problem inputs and returns [L, B, 1024] fp32.
"""

import numpy as np

import concourse.bass as bass
import concourse.bacc as bacc
import concourse.tile as tile
from concourse import mybir
from concourse.bass_utils import run_bass_kernel_spmd

F16 = mybir.dt.float16
F32 = mybir.dt.float32
F32R = mybir.dt.float32r

L = 128
B = 64
NCORES = 8
BL = 16          # batch per core
E = 512          # embedding dim
H = 512          # hidden per direction
G = 2048         # 4*H gates
NW = 32000

AF = mybir.ActivationFunctionType


def build_program():
    nc = bacc.Bacc(trn_type="TRN2")

    # rows 0..511: embT; row 512: ones (bias); row 513: (m-1) (mask fold)
    xt = nc.declare_dram_parameter("xt", [E + 2, L * BL], F16, isOutput=False)
    # rows 0..511: WihT; row 512: bias; row 513: 60 on i,f gate blocks
    wih = nc.declare_dram_parameter("wih", [E + 2, G], F16, isOutput=False)
    whh = nc.declare_dram_parameter("whh", [H, G], F16, isOutput=False)           # WhhT
    eye = nc.declare_dram_parameter("eye", [16, 16], F32, isOutput=False)         # identity 16
    hs = nc.declare_dram_parameter("hs", [L * BL, H], F32, isOutput=True)

    xw_dram = nc.dram_tensor("xw_scratch", [L * BL, G], F32R)

    with tile.TileContext(nc) as tc:
        with (
            tc.tile_pool(name="consts", bufs=1) as consts,
            tc.tile_pool(name="prepsum", bufs=2, space="PSUM") as prepsum,
            tc.tile_pool(name="evac", bufs=3) as evac,
            tc.tile_pool(name="ring", bufs=4) as ring,
            tc.tile_pool(name="gates", bufs=1, space="PSUM") as gatesp,
            tc.tile_pool(name="tpsum", bufs=2, space="PSUM") as tpsum,
            tc.tile_pool(name="acts", bufs=2) as acts,
            tc.tile_pool(name="state", bufs=2) as state,
        ):
            # ---- load constants to SBUF ----
            w_x = consts.tile([128, 4 * L * BL], F16)     # embT chunks (k major)
            w_ih = consts.tile([128, 4 * G], F16)
            w_hh = consts.tile([128, 4 * G], F16)
            ones_x = consts.tile([2, L * BL], F16)
            bias_w = consts.tile([2, G], F16)
            eye_sb = consts.tile([16, 16], F32)
            LB = L * BL
            for k in range(4):
                nc.sync.dma_start(out=w_x[:, k * LB:(k + 1) * LB],
                                  in_=xt[128 * k:128 * (k + 1), :])
                nc.sync.dma_start(out=w_ih[:, k * G:(k + 1) * G],
                                  in_=wih[128 * k:128 * (k + 1), :])
                nc.sync.dma_start(out=w_hh[:, k * G:(k + 1) * G],
                                  in_=whh[128 * k:128 * (k + 1), :])
            nc.sync.dma_start(out=ones_x, in_=xt[E:E + 2, :])
            nc.sync.dma_start(out=bias_w, in_=wih[E:E + 2, :])
            nc.sync.dma_start(out=eye_sb, in_=eye[:, :])
            eye_r = consts.tile([16, 16], F32R)
            nc.vector.tensor_copy(eye_r, eye_sb)   # rounded copy for fp32r matmul

            # ---- precompute xw = x @ WihT + bias (+ mask fold on i,f) ----
            for c in range(16):          # LB chunks of 128 rows
                for n in range(4):       # gate tiles of 512
                    pp = prepsum.tile([128, 512], F32)
                    for k in range(4):
                        nc.tensor.matmul(
                            pp,
                            lhsT=w_x[:, k * LB + 128 * c: k * LB + 128 * (c + 1)],
                            rhs=w_ih[:, k * G + 512 * n: k * G + 512 * (n + 1)],
                            start=(k == 0), stop=False)
                    nc.tensor.matmul(
                        pp,
                        lhsT=ones_x[:, 128 * c:128 * (c + 1)],
                        rhs=bias_w[:, 512 * n:512 * (n + 1)],
                        start=False, stop=True)  # K=2: bias + mask rows
                    ev = evac.tile([128, 512], F32R)
                    nc.vector.tensor_copy(ev, pp)
                    nc.gpsimd.dma_start(
                        out=xw_dram[128 * c:128 * (c + 1), 512 * n:512 * (n + 1)],
                        in_=ev)

            # ---- recurrence state ----
            hT_prev = state.tile([128, 64], F16, tag="hT")
            c_prev = state.tile([16, H], F32, tag="c")
            nc.vector.memset(hT_prev, 0.0)
            nc.vector.memset(c_prev, 0.0)

            for t in range(L):
                xwt = ring.tile([16, G], F32R, tag="xwt")
                nc.gpsimd.dma_start(out=xwt, in_=xw_dram[16 * t:16 * (t + 1), :])

                pg = gatesp.tile([16, G], F32, tag="pg")
                for n in range(4):
                    sl = slice(512 * n, 512 * (n + 1))
                    nc.tensor.matmul(
                        pg[:, sl],
                        lhsT=eye_r,
                        rhs=xwt[:, sl],
                        start=True, stop=False)
                    for k in range(4):
                        nc.tensor.matmul(
                            pg[:, sl],
                            lhsT=hT_prev[:, 16 * k:16 * (k + 1)],
                            rhs=w_hh[:, k * G + 512 * n: k * G + 512 * (n + 1)],
                            start=False, stop=(k == 3))

                tg = acts.tile([16, H], F32, tag="tg")
                nc.scalar.activation(tg, pg[:, 0:512], AF.Tanh)
                sig = acts.tile([16, 3 * H], F32, tag="sig")
                nc.scalar.activation(sig, pg[:, 512:2048], AF.Sigmoid)

                u = acts.tile([16, H], F32, tag="u")
                nc.vector.tensor_mul(u, sig[:, 0:512], tg)          # sig(i)*tanh(g)
                v = acts.tile([16, H], F32, tag="v")
                nc.vector.tensor_mul(v, sig[:, 512:1024], c_prev)   # sig(f)*c
                c_new = state.tile([16, H], F32, tag="c")
                nc.vector.tensor_add(c_new, u, v)

                tc_t = acts.tile([16, H], F32, tag="tc")
                nc.scalar.activation(tc_t, c_new, AF.Tanh)
                h_t = acts.tile([16, H], F32, tag="h")
                nc.vector.tensor_mul(h_t, sig[:, 1024:1536], tc_t)  # sig(o)*tanh(c)

                nc.gpsimd.dma_start(out=hs[16 * t:16 * (t + 1), :], in_=h_t)

                pT = tpsum.tile([128, 64], F32, tag="pT")
                for k in range(4):
                    nc.tensor.matmul(
                        pT[:, 16 * k:16 * (k + 1)],
                        lhsT=h_t[:, 128 * k:128 * (k + 1)],
                        rhs=eye_sb,
                        is_transpose=True,
                        start=True, stop=True, skip_group_check=True)
                hT_new = state.tile([128, 64], F16, tag="hT")
                nc.vector.tensor_copy(hT_new, pT)

                hT_prev, c_prev = hT_new, c_new

    nc.compile()
    return nc


_NC_CACHE = None


def _get_program():
    global _NC_CACHE
    if _NC_CACHE is None:
        _NC_CACHE = build_program()
    return _NC_CACHE


def _prep_core_inputs(emb, msk, WihT_r, WhhT_r, bias_r):
    """emb [L, BL, E] f32 (already direction-ordered), msk [L, BL] f32."""
    x2 = emb.reshape(L * BL, E)
    mflat = msk.reshape(1, L * BL)
    xt = np.concatenate(
        [x2.T, np.ones((1, L * BL), np.float32), mflat - 1.0], axis=0)
    maskw = np.zeros((1, G), np.float32)
    maskw[0, 512:1536] = 60.0            # i,f blocks in [g|i|f|o] order
    wih_aug = np.concatenate([WihT_r, bias_r[None, :], maskw], axis=0)
    return {
        "xt": xt.astype(np.float16),
        "wih": wih_aug.astype(np.float16),
        "whh": WhhT_r.astype(np.float16),
        "eye": np.eye(16, dtype=np.float32),
    }


def kernel(**inputs):
    data = np.asarray(inputs["data"])
    mask = np.asarray(inputs["mask"], dtype=np.float32)
    emb_table = np.asarray(inputs["emb_table"], dtype=np.float32)

    q = np.where(data == -1, NW, data).astype(np.int64)
    emb = emb_table[q]                        # [L, B, E]
    emb[q == NW] = 0.0

    perm = np.concatenate([
        np.arange(1024, 1536),   # g
        np.arange(0, 512),       # i
        np.arange(512, 1024),    # f
        np.arange(1536, 2048),   # o
    ])

    dirs = []
    for d in ("f", "b"):
        WihT = np.asarray(inputs[f"Wih_{d}"], np.float32).T[:, perm]
        WhhT = np.asarray(inputs[f"Whh_{d}"], np.float32).T[:, perm]
        bias = (np.asarray(inputs[f"bih_{d}"], np.float32)
                + np.asarray(inputs[f"bhh_{d}"], np.float32))[perm]
        dirs.append((WihT, WhhT, bias))

    in_maps = []
    for core in range(NCORES):
        d = 0 if core < 4 else 1
        qtr = core % 4
        cols = slice(16 * qtr, 16 * (qtr + 1))
        emb_c = emb[:, cols, :]
        msk_c = mask[:, cols]
        if d == 1:
            emb_c = emb_c[::-1]
            msk_c = msk_c[::-1]
        WihT, WhhT, bias = dirs[d]
        in_maps.append(_prep_core_inputs(np.ascontiguousarray(emb_c),
                                         np.ascontiguousarray(msk_c),
                                         WihT, WhhT, bias))

    nc = _get_program()
    import os
    kw = {}
    if os.environ.get("BASS_KERNEL_TRACE", "0") == "1":
        kw = dict(trace=True, tmpdir=os.environ.get("BASS_KERNEL_TRACE_DIR"))
    res = run_bass_kernel_spmd(nc, in_maps, list(range(NCORES)), **kw)
    global LAST_RESULTS
    LAST_RESULTS = res
    if res.exec_time_ns is not None:
        print(f"HW exec time: {res.exec_time_ns} ns")
    outs = [r["hs"].reshape(L, BL, H) for r in res.results]

    fwd = np.concatenate(outs[0:4], axis=1)          # [L, B, H]
    bwd = np.concatenate(outs[4:8], axis=1)[::-1]    # un-reverse time
    return np.concatenate([fwd, bwd], axis=-1).astype(np.float32)
